# revision 9
# baseline (speedup 1.0000x reference)
"""BERT-style single-layer transformer on 8 Trainium2 NeuronCores.

Sharding: data-parallel over batch (2 batches/core) for embedding + LN +
attention; the lm_head is vocab-sharded (4000 vocab cols/core) with an
on-device AllGather of the gathered masked positions.

Per-core device program (all fp32):
  - token embedding via indirect-DMA row gather; segment embedding gathered
    with DMA-accumulate; position embedding DMA-accumulated on top
  - LayerNorm (bn_stats) in natural [t, d] layout, PE transpose to
    xT [d-chunk, t]
  - QKV: qT/kT per head-pair [128, 512], v natural [t, 12*65] with a ones
    column per head (so the PV matmul also produces softmax denominators)
  - scores S^T = k @ q^T in [key, query] layout; query mask (*1/8) folded
    into the qT PSUM->SBUF copy; max-free softmax via ACT Exp (scores are
    tiny: layernormed x times 0.02-scale weights)
  - PV natural + per-partition normalize -> x_att [t, d]
  - one-hot selection matmul -> xmT [d, 21] per batch (masked pos + cls)
  - AllGather xmT across 8 cores, lm_head over the local vocab shard
"""

import numpy as np

import concourse.bass as bass
import concourse.tile as tile
from concourse import bacc, mybir
from concourse.bass import IndirectOffsetOnAxis
from concourse.bass_utils import run_bass_kernel_spmd
from concourse.masks import make_identity

F32 = mybir.dt.float32
I32 = mybir.dt.int32

V, NSEG, MAXLEN, D, H = 32000, 2, 512, 768, 12
HS = D // H                     # 64
B, T, NMASK = 16, 512, 20
LN_EPS = 1e-5
N_CORES = 8
BPC = B // N_CORES              # batches per core = 2
VS = V // N_CORES               # vocab shard = 4000
P = 128
KD = D // P                     # d chunks = 6
TT = T // P                     # t tiles per batch = 4
NPAIR = H // 2                  # head pairs = 6
NM1 = NMASK + 1                 # masked positions + cls column = 21
MROWS = B * NMASK               # 320 lm rows
VT_SIZES = [512] * 7 + [VS - 512 * 7]   # vocab tiles per core


def build_program(spmd=True):
    nc = bacc.Bacc("TRN2", target_bir_lowering=False, debug=False,
                   num_devices=N_CORES if spmd else 1)

    def din(name, shape, dtype=F32):
        return nc.dram_tensor(name, shape, dtype, kind="ExternalInput").ap()

    tok_idx = din("tok_idx", [P, BPC * TT], I32)     # col g = b*TT+ti
    seg_idx = din("seg_idx", [P, BPC * TT], I32)
    mask_sc = din("mask_sc", [BPC, T])               # attn_mask * 1/8
    sel = din("sel", [P, BPC * TT * NM1])            # one-hot select [p, b, ti, m]
    pos_emb = din("pos_emb", [MAXLEN, D])
    tok_emb = din("tok_emb", [V, D])
    seg_emb = din("seg_emb", [NSEG, D])
    ln_g = din("ln_g", [1, D])
    ln_b = din("ln_b", [1, D])
    wq = din("wq", [D, D])                           # [d, h*hs]
    wk = din("wk", [D, D])
    wv = din("wv", [D, D])
    w_lm = din("w_lm", [D, VS])
    b_lm = din("b_lm", [1, VS])
    w_cls = din("w_cls", [P, KD * 2])                # prearranged [p, ko, 2]
    b_cls = din("b_cls", [1, 2])

    out_lm = nc.dram_tensor("out_lm", [MROWS, VS], F32, kind="ExternalOutput").ap()
    out_cls = nc.dram_tensor("out_cls", [BPC, 2], F32, kind="ExternalOutput").ap()

    with tile.TileContext(nc) as tc:
        _emit(tc, locals(), spmd)
    nc.compile()
    return nc


def _emit(tc, t, spmd=True):
    nc = tc.nc
    AF = mybir.ActivationFunctionType
    OP = mybir.AluOpType

    with tc.tile_pool(name="const", bufs=1) as constp, \
         tc.tile_pool(name="qkv", bufs=1) as qkvp, \
         tc.tile_pool(name="attout", bufs=1) as attoutp, \
         tc.tile_pool(name="dram", bufs=1, space="DRAM") as dramp:

        # ---------------- constants ----------------
        ident = constp.tile([P, P], F32)
        make_identity(nc, ident[:])
        tok_idx_sb = constp.tile([P, BPC * TT], I32)
        nc.sync.dma_start(tok_idx_sb[:], t["tok_idx"][:])
        seg_idx_sb = constp.tile([P, BPC * TT], I32)
        nc.sync.dma_start(seg_idx_sb[:], t["seg_idx"][:])
        mask_b = constp.tile([P, BPC, T], F32)
        for b in range(BPC):
            nc.sync.dma_start(mask_b[:, b, :],
                              t["mask_sc"][b:b + 1, :].to_broadcast([P, T]))
        sel_sb = constp.tile([P, BPC, TT, NM1], F32)
        nc.sync.dma_start(
            sel_sb[:].rearrange("p b ti m -> p (b ti m)"), t["sel"][:])
        g_b = constp.tile([P, D], F32)
        nc.sync.dma_start(g_b[:], t["ln_g"][:].to_broadcast([P, D]))
        b_b = constp.tile([P, D], F32)
        nc.sync.dma_start(b_b[:], t["ln_b"][:].to_broadcast([P, D]))
        eps_sb = constp.tile([P, 1], F32)
        nc.vector.memset(eps_sb[:], LN_EPS)
        w_cls_sb = constp.tile([P, KD, 2], F32)
        nc.sync.dma_start(w_cls_sb[:].rearrange("p k c -> p (k c)"), t["w_cls"][:])
        b_cls_sb = constp.tile([1, 2], F32)
        nc.sync.dma_start(b_cls_sb[:], t["b_cls"][:])

        # persistent activations
        q_sb = qkvp.tile([P, BPC, NPAIR, T], F32)
        k_sb = qkvp.tile([P, BPC, NPAIR, T], F32)
        v_sb = qkvp.tile([P, BPC, TT, H * (HS + 1)], F32)   # ones col per head
        x_att = attoutp.tile([P, BPC, TT, D], F32)

        wq_r = t["wq"].rearrange("(ko p) m -> p ko m", p=P)
        wk_r = t["wk"].rearrange("(ko p) m -> p ko m", p=P)
        wv_r = t["wv"].rearrange("(ko p) m -> p ko m", p=P)
        pos_r = t["pos_emb"].rearrange("(ti p) d -> ti p d", p=P)

        # ---------------- phase 1: embed + LN + transpose + QKV ----------
        with tc.tile_pool(name="p1w", bufs=1) as p1w, \
             tc.tile_pool(name="p1t", bufs=3) as p1t, \
             tc.tile_pool(name="ps1", bufs=2, space="PSUM") as ps1, \
             tc.tile_pool(name="psqk", bufs=2, space="PSUM") as psqk, \
             tc.tile_pool(name="psv", bufs=2, space="PSUM") as psv:

            wq_sb = p1w.tile([P, KD, D], F32)
            nc.sync.dma_start(wq_sb[:], wq_r)
            wk_sb = p1w.tile([P, KD, D], F32)
            nc.sync.dma_start(wk_sb[:], wk_r)
            wv_sb = p1w.tile([P, KD, D], F32)
            nc.sync.dma_start(wv_sb[:], wv_r)

            for b in range(BPC):
                xT = p1w.tile([P, KD, T], F32, tag="xT")
                for ti in range(TT):
                    g = b * TT + ti
                    emb = p1t.tile([P, D], F32, tag="emb")
                    nc.gpsimd.indirect_dma_start(
                        out=emb[:], out_offset=None, in_=t["tok_emb"][:],
                        in_offset=IndirectOffsetOnAxis(
                            ap=tok_idx_sb[:, g:g + 1], axis=0))
                    nc.gpsimd.indirect_dma_start(
                        out=emb[:], out_offset=None, in_=t["seg_emb"][:],
                        in_offset=IndirectOffsetOnAxis(
                            ap=seg_idx_sb[:, g:g + 1], axis=0),
                        compute_op=OP.add)
                    nc.gpsimd.dma_start(out=emb[:], in_=pos_r[ti],
                                        accum_op=OP.add)
                    # LayerNorm over d
                    stats = p1t.tile([P, 3, 6], F32, tag="stats")
                    for w in range(3):
                        nc.vector.bn_stats(out=stats[:, w, :],
                                           in_=emb[:, 256 * w:256 * (w + 1)])
                    mv = p1t.tile([P, 2], F32, tag="mv")
                    nc.vector.bn_aggr(out=mv[:], in_=stats[:])
                    rstd = p1t.tile([P, 1], F32, tag="rstd")
                    nc.scalar.activation(out=rstd[:], in_=mv[:, 1:2],
                                         func=AF.Sqrt, bias=eps_sb[:])
                    nc.vector.reciprocal(out=rstd[:], in_=rstd[:])
                    xln = p1t.tile([P, D], F32, tag="xln")
                    nc.vector.tensor_scalar(
                        out=xln[:], in0=emb[:], scalar1=mv[:, 0:1],
                        scalar2=rstd[:], op0=OP.subtract, op1=OP.mult)
                    nc.vector.tensor_mul(xln[:], xln[:], g_b[:])
                    nc.gpsimd.tensor_tensor(
                        out=xln[:], in0=xln[:], in1=b_b[:], op=OP.add)
                    for j in range(KD):
                        pst = ps1.tile([P, P], F32, tag="tr")
                        nc.tensor.transpose(pst[:], xln[:, P * j:P * (j + 1)],
                                            ident[:])
                        nc.scalar.copy(out=xT[:, j, P * ti:P * (ti + 1)],
                                       in_=pst[:])

                # q/k projections: per head pair [128, 512]
                for pr in range(NPAIR):
                    psq = psqk.tile([P, T], F32, tag="qk")
                    for ki in range(KD):
                        nc.tensor.matmul(
                            psq[:], lhsT=wq_sb[:, ki, P * pr:P * (pr + 1)],
                            rhs=xT[:, ki, :], start=(ki == 0), stop=(ki == KD - 1))
                    # fold query mask * 1/8 into the copy-out
                    nc.vector.tensor_mul(q_sb[:, b, pr, :], psq[:], mask_b[:, b, :])
                    psk = psqk.tile([P, T], F32, tag="qk")
                    for ki in range(KD):
                        nc.tensor.matmul(
                            psk[:], lhsT=wk_sb[:, ki, P * pr:P * (pr + 1)],
                            rhs=xT[:, ki, :], start=(ki == 0), stop=(ki == KD - 1))
                    nc.scalar.copy(out=k_sb[:, b, pr, :], in_=psk[:])

                # v natural [t, h*hs] + ones columns
                for ti in range(TT):
                    pv = psv.tile([P, D], F32, tag="v")
                    for ki in range(KD):
                        nc.tensor.matmul(
                            pv[:, 0:512], lhsT=xT[:, ki, P * ti:P * (ti + 1)],
                            rhs=wv_sb[:, ki, 0:512],
                            start=(ki == 0), stop=(ki == KD - 1))
                    for ki in range(KD):
                        nc.tensor.matmul(
                            pv[:, 512:768], lhsT=xT[:, ki, P * ti:P * (ti + 1)],
                            rhs=wv_sb[:, ki, 512:768],
                            start=(ki == 0), stop=(ki == KD - 1))
                    vv = v_sb[:, b, ti, :].rearrange("p (h s) -> p h s", s=HS + 1)
                    nc.vector.memset(v_sb[:, b, ti, :], 1.0)
                    nc.vector.tensor_copy(
                        out=vv[:, :, 0:HS],
                        in_=pv[:].rearrange("p (h s) -> p h s", s=HS))

        # ------------- phase 2+3: attention, select, allgather, heads ----
        with tc.tile_pool(name="att", bufs=2) as attp, \
             tc.tile_pool(name="attr", bufs=8) as attrp, \
             tc.tile_pool(name="xm", bufs=1) as xmp, \
             tc.tile_pool(name="lmw", bufs=2) as lmwp, \
             tc.tile_pool(name="lmc", bufs=1) as lmcp, \
             tc.tile_pool(name="lmo", bufs=3) as lmop, \
             tc.tile_pool(name="pss", bufs=4, space="PSUM") as pss_p, \
             tc.tile_pool(name="psa", bufs=4, space="PSUM") as psa_p:

            for b in range(BPC):
                for h in range(H):
                    pr, off = h // 2, HS * (h % 2)
                    pt = attp.tile([P, TT, T], F32, tag="pt")
                    for u in range(TT):
                        pss = pss_p.tile([P, T], F32, tag="s")
                        nc.tensor.matmul(
                            pss[:],
                            lhsT=k_sb[off:off + HS, b, pr, P * u:P * (u + 1)],
                            rhs=q_sb[off:off + HS, b, pr, :],
                            start=True, stop=True)
                        nc.scalar.activation(out=pt[:, u, :], in_=pss[:],
                                             func=AF.Exp)
                    vv = v_sb[:, b, :, :].rearrange("p ti (h s) -> p ti h s",
                                                    s=HS + 1)
                    for ti in range(TT):
                        psa = psa_p.tile([P, HS + 1], F32, tag="pv")
                        for u in range(TT):
                            nc.tensor.matmul(
                                psa[:], lhsT=pt[:, u, P * ti:P * (ti + 1)],
                                rhs=vv[:, u, h, :],
                                start=(u == 0), stop=(u == TT - 1))
                        rcp = attrp.tile([P, 1], F32, tag="rcp")
                        nc.vector.reciprocal(out=rcp[:], in_=psa[:, HS:HS + 1])
                        nc.vector.tensor_scalar_mul(
                            out=x_att[:, b, ti, HS * h:HS * (h + 1)],
                            in0=psa[:, 0:HS], scalar1=rcp[:])

            # ---- select masked positions (+ cls col) via one-hot matmul ----
            xm_sb = xmp.tile([P, BPC, KD, NM1], F32)
            for b in range(BPC):
                for j in range(KD):
                    psx = psa_p.tile([P, NM1], F32, tag="pv")
                    for ti in range(TT):
                        nc.tensor.matmul(
                            psx[:], lhsT=x_att[:, b, ti, P * j:P * (j + 1)],
                            rhs=sel_sb[:, b, ti, :],
                            start=(ti == 0), stop=(ti == TT - 1))
                    nc.scalar.copy(out=xm_sb[:, b, j, :], in_=psx[:])

            # cls head (local batches only)
            for b in range(BPC):
                psc = psa_p.tile([1, 2], F32, tag="pv")
                for j in range(KD):
                    nc.tensor.matmul(
                        psc[:], lhsT=xm_sb[:, b, j, NMASK:NM1],
                        rhs=w_cls_sb[:, j, :],
                        start=(j == 0), stop=(j == KD - 1))
                ocls = xmp.tile([1, 2], F32, tag="ocls")
                nc.vector.tensor_add(ocls[:], psc[:], b_cls_sb[:])
                nc.sync.dma_start(t["out_cls"][b:b + 1, :], ocls[:])

            # allgather masked-position features
            ag_in = dramp.tile([BPC, KD, P, NM1], F32)
            ag_out = dramp.tile([N_CORES, BPC, KD, P, NM1], F32)
            nc.sync.dma_start(ag_in[:].rearrange("b c p m -> p (b c) m"),
                              xm_sb[:].rearrange("p b c m -> p (b c) m"))
            if spmd:
                nc.gpsimd.collective_compute(
                    "AllGather", mybir.AluOpType.bypass,
                    replica_groups=[list(range(N_CORES))],
                    ins=[ag_in.opt()], outs=[ag_out.opt()])
            else:
                # timing-only stand-in for the AllGather: replicate the local
                # contribution so downstream reads see equivalent traffic
                for r in range(N_CORES):
                    nc.sync.dma_start(ag_out[r], ag_in[:])

            xg_sb = lmcp.tile([P, KD, MROWS], F32)
            agv = ag_out[:].rearrange("r b c p m -> (r b) c p m")
            for j in range(KD):
                nc.sync.dma_start(
                    xg_sb[:, j, :].rearrange("p (g m) -> p g m", m=NMASK),
                    agv[:, j, :, 0:NMASK].rearrange("g p m -> p g m"))

            # lm head over local vocab shard
            blm_b = lmcp.tile([P, VS], F32)
            nc.sync.dma_start(blm_b[:], t["b_lm"][:].to_broadcast([P, VS]))
            wlm_r = t["w_lm"].rearrange("(ko p) v -> p ko v", p=P)
            voff = 0
            for vt, nv in enumerate(VT_SIZES):
                wt = lmwp.tile([P, KD, 512], F32, tag="wlm")
                nc.sync.dma_start(wt[:, :, 0:nv], wlm_r[:, :, voff:voff + nv])
                for mt in range(3):
                    mp = min(P, MROWS - P * mt)
                    pso = pss_p.tile([P, 512], F32, tag="s")
                    for j in range(KD):
                        nc.tensor.matmul(
                            pso[:mp, 0:nv], lhsT=xg_sb[:, j, P * mt:P * mt + mp],
                            rhs=wt[:, j, 0:nv],
                            start=(j == 0), stop=(j == KD - 1))
                    osb = lmop.tile([P, 512], F32, tag="osb")
                    nc.vector.tensor_add(osb[:mp, 0:nv], pso[:mp, 0:nv],
                                         blm_b[:mp, voff:voff + nv])
                    nc.sync.dma_start(
                        t["out_lm"][P * mt:P * mt + mp, voff:voff + nv],
                        osb[:mp, 0:nv])
                voff += nv


_CACHE = {}


def _get_program():
    if "nc" not in _CACHE:
        _CACHE["nc"] = build_program()
    return _CACHE["nc"]


def _prep_inputs(inputs):
    seq = np.ascontiguousarray(np.asarray(inputs["sequence"], np.int64)).astype(np.int32)
    seg = np.ascontiguousarray(np.asarray(inputs["segment"], np.int64)).astype(np.int32)
    amask = np.asarray(inputs["attn_mask"]).astype(np.float32)
    mpos = np.asarray(inputs["masked_pos"], np.int64).astype(np.int64)
    f = lambda k: np.ascontiguousarray(np.asarray(inputs[k], np.float32))
    tok_emb, seg_emb, pos_emb = f("tok_emb"), f("seg_emb"), f("pos_emb")
    ln_g, ln_b = f("ln_g").reshape(1, D), f("ln_b").reshape(1, D)
    Wq, Wk, Wv = f("Wq"), f("Wk"), f("Wv")
    W_lm, b_lm = f("W_lm"), f("b_lm").reshape(1, V)
    W_cls, b_cls = f("W_cls"), f("b_cls").reshape(1, 2)

    wq_flat = np.ascontiguousarray(Wq.transpose(1, 0, 2).reshape(D, D))
    wk_flat = np.ascontiguousarray(Wk.transpose(1, 0, 2).reshape(D, D))
    wv_flat = np.ascontiguousarray(Wv.transpose(1, 0, 2).reshape(D, D))
    # [768, 2] -> [6, 128, 2] -> [128, 6, 2] -> [128, 12]
    w_cls_dev = np.ascontiguousarray(
        W_cls.reshape(KD, P, 2).transpose(1, 0, 2).reshape(P, KD * 2))

    in_maps = []
    for c in range(N_CORES):
        b0 = BPC * c
        # [2, 512] -> [2*4, 128] -> [128, 8]
        tok_idx = np.ascontiguousarray(
            seq[b0:b0 + BPC].reshape(BPC * TT, P).T)
        seg_idx = np.ascontiguousarray(
            seg[b0:b0 + BPC].reshape(BPC * TT, P).T)
        mask_sc = np.ascontiguousarray(amask[b0:b0 + BPC] * np.float32(HS ** -0.5))
        sel = np.zeros((BPC, T, NM1), np.float32)
        for b in range(BPC):
            sel[b, mpos[b0 + b], np.arange(NMASK)] = 1.0
            sel[b, 0, NMASK] = 1.0
        # [2, 512, 21] -> [2, 4, 128, 21] -> [128, 2, 4, 21] -> [128, 168]
        sel_dev = np.ascontiguousarray(
            sel.reshape(BPC, TT, P, NM1).transpose(2, 0, 1, 3).reshape(P, -1))
        in_maps.append({
            "tok_idx": tok_idx, "seg_idx": seg_idx, "mask_sc": mask_sc,
            "sel": sel_dev, "pos_emb": pos_emb, "tok_emb": tok_emb,
            "seg_emb": seg_emb, "ln_g": ln_g, "ln_b": ln_b,
            "wq": wq_flat, "wk": wk_flat, "wv": wv_flat,
            "w_lm": np.ascontiguousarray(W_lm[:, VS * c:VS * (c + 1)]),
            "b_lm": np.ascontiguousarray(b_lm[:, VS * c:VS * (c + 1)]),
            "w_cls": w_cls_dev, "b_cls": b_cls,
        })
    return in_maps


def kernel(**inputs):
    nc = _get_program()
    in_maps = _prep_inputs(inputs)
    res = run_bass_kernel_spmd(nc, in_maps, core_ids=list(range(N_CORES)))
    logits_lm = np.empty((B, NMASK, V), np.float32)
    logits_clsf = np.empty((B, 2), np.float32)
    for c in range(N_CORES):
        logits_lm[:, :, VS * c:VS * (c + 1)] = \
            res.results[c]["out_lm"].reshape(B, NMASK, VS)
        logits_clsf[BPC * c:BPC * (c + 1)] = res.results[c]["out_cls"]
    return (logits_lm, logits_clsf)


# revision 16
# speedup vs baseline: 1.9188x; 1.9188x over previous
"""BERT-style single-layer transformer on 8 Trainium2 NeuronCores.

Sharding: data-parallel over batch (2 batches/core) for embedding + LN +
attention; the lm_head is vocab-sharded (4000 vocab cols/core) with an
on-device AllGather of the gathered masked positions.

Per-core device program (all fp32):
  - token embedding via indirect-DMA row gather; segment embedding gathered
    with DMA-accumulate; position embedding DMA-accumulated on top
  - LayerNorm (bn_stats) in natural [t, d] layout, PE transpose to
    xT [d-chunk, t]
  - QKV: qT/kT per head-pair [128, 512], v natural [t, 12*65] with a ones
    column per head (so the PV matmul also produces softmax denominators)
  - scores S^T = k @ q^T in [key, query] layout; query mask (*1/8) folded
    into the qT PSUM->SBUF copy; max-free softmax via ACT Exp (scores are
    tiny: layernormed x times 0.02-scale weights)
  - PV natural + per-partition normalize -> x_att [t, d]
  - one-hot selection matmul -> xmT [d, 21] per batch (masked pos + cls)
  - AllGather xmT across 8 cores, lm_head over the local vocab shard
"""

import numpy as np

import concourse.bass as bass
import concourse.tile as tile
from concourse import bacc, mybir
from concourse.bass import IndirectOffsetOnAxis
from concourse.bass_utils import run_bass_kernel_spmd
from concourse.masks import make_identity

F32 = mybir.dt.float32
F32R = mybir.dt.float32r
I32 = mybir.dt.int32

V, NSEG, MAXLEN, D, H = 32000, 2, 512, 768, 12
HS = D // H                     # 64
B, T, NMASK = 16, 512, 20
LN_EPS = 1e-5
N_CORES = 8
BPC = B // N_CORES              # batches per core = 2
VS = V // N_CORES               # vocab shard = 4000
P = 128
KD = D // P                     # d chunks = 6
TT = T // P                     # t tiles per batch = 4
NPAIR = H // 2                  # head pairs = 6
NM1 = NMASK + 1                 # masked positions + cls column = 21
MROWS = B * NMASK               # 320 lm rows
VT_SIZES = [512] * 7 + [VS - 512 * 7]   # vocab tiles per core


def build_program(spmd=True):
    nc = bacc.Bacc("TRN2", target_bir_lowering=False, debug=False,
                   num_devices=N_CORES if spmd else 1)

    def din(name, shape, dtype=F32):
        return nc.dram_tensor(name, shape, dtype, kind="ExternalInput").ap()

    tok_idx = din("tok_idx", [P, BPC * TT], I32)     # col g = b*TT+ti
    seg_idx = din("seg_idx", [P, BPC * TT], I32)
    mask_sc = din("mask_sc", [BPC, T])               # attn_mask * 1/8
    sel = din("sel", [P, BPC * TT * NM1])            # one-hot select [p, b, ti, m]
    pos_emb = din("pos_emb", [MAXLEN, D])
    tok_emb = din("tok_emb", [V, D])
    seg_emb = din("seg_emb", [NSEG, D])
    ln_g = din("ln_g", [1, D])
    ln_b = din("ln_b", [1, D])
    wq = din("wq", [D, D])                           # [d, h*hs]
    wk = din("wk", [D, D])
    wv = din("wv", [D, D])
    w_lm = din("w_lm", [D, VS])
    b_lm = din("b_lm", [1, VS])
    w_cls = din("w_cls", [P, KD * 2])                # prearranged [p, ko, 2]
    b_cls = din("b_cls", [1, 2])

    out_lm = nc.dram_tensor("out_lm", [MROWS, VS], F32, kind="ExternalOutput").ap()
    out_cls = nc.dram_tensor("out_cls", [BPC, 2], F32, kind="ExternalOutput").ap()

    with tile.TileContext(nc) as tc:
        _emit(tc, locals(), spmd)
    nc.compile()
    return nc


def _emit(tc, t, spmd=True):
    nc = tc.nc
    AF = mybir.ActivationFunctionType
    OP = mybir.AluOpType

    with tc.tile_pool(name="const", bufs=1) as constp, \
         tc.tile_pool(name="qkv", bufs=1) as qkvp, \
         tc.tile_pool(name="attout", bufs=1) as attoutp, \
         tc.tile_pool(name="dram", bufs=1, space="DRAM") as dramp:

        # ---------------- constants ----------------
        ident = constp.tile([P, P], F32)
        make_identity(nc, ident[:])
        tok_idx_sb = constp.tile([P, BPC * TT], I32)
        nc.sync.dma_start(tok_idx_sb[:], t["tok_idx"][:])
        seg_idx_sb = constp.tile([P, BPC * TT], I32)
        nc.sync.dma_start(seg_idx_sb[:], t["seg_idx"][:])
        mask_b = constp.tile([P, BPC, T], F32)
        for b in range(BPC):
            nc.sync.dma_start(mask_b[:, b, :],
                              t["mask_sc"][b:b + 1, :].to_broadcast([P, T]))
        sel_sb = constp.tile([P, BPC, TT, NM1], F32)
        nc.sync.dma_start(
            sel_sb[:].rearrange("p b ti m -> p (b ti m)"), t["sel"][:])
        g_b = constp.tile([P, D], F32)
        nc.sync.dma_start(g_b[:], t["ln_g"][:].to_broadcast([P, D]))
        b_b = constp.tile([P, D], F32)
        nc.sync.dma_start(b_b[:], t["ln_b"][:].to_broadcast([P, D]))
        eps_sb = constp.tile([P, 1], F32)
        nc.vector.memset(eps_sb[:], LN_EPS)
        w_cls_sb = constp.tile([P, KD, 2], F32)
        nc.sync.dma_start(w_cls_sb[:].rearrange("p k c -> p (k c)"), t["w_cls"][:])
        b_cls_sb = constp.tile([1, 2], F32)
        nc.sync.dma_start(b_cls_sb[:], t["b_cls"][:])

        # persistent activations
        q_sb = qkvp.tile([P, BPC, NPAIR, T], F32R)
        k_sb = qkvp.tile([P, BPC, NPAIR, T], F32R)
        v_sb = qkvp.tile([P, BPC, TT, H * (HS + 1)], F32)   # ones col per head
        x_att = attoutp.tile([P, BPC, TT, D], F32)

        wq_r = t["wq"].rearrange("(ko p) m -> p ko m", p=P)
        wk_r = t["wk"].rearrange("(ko p) m -> p ko m", p=P)
        wv_r = t["wv"].rearrange("(ko p) m -> p ko m", p=P)
        pos_r = t["pos_emb"].rearrange("(ti p) d -> ti p d", p=P)

        # ---------------- phase 1: embed + LN + transpose + QKV ----------
        with tc.tile_pool(name="p1w", bufs=1) as p1w, \
             tc.tile_pool(name="p1t", bufs=3) as p1t, \
             tc.tile_pool(name="ps1", bufs=2, space="PSUM") as ps1, \
             tc.tile_pool(name="psqk", bufs=2, space="PSUM") as psqk, \
             tc.tile_pool(name="psv", bufs=2, space="PSUM") as psv:

            wq_sb = p1w.tile([P, KD, D], F32R)
            nc.gpsimd.dma_start(out=wq_sb[:], in_=wq_r)
            wk_sb = p1w.tile([P, KD, D], F32R)
            nc.gpsimd.dma_start(out=wk_sb[:], in_=wk_r)
            wv_sb = p1w.tile([P, KD, D], F32R)
            nc.gpsimd.dma_start(out=wv_sb[:], in_=wv_r)

            for b in range(BPC):
                xT = p1w.tile([P, KD, T], F32R, tag="xT")
                for ti in range(TT):
                    g = b * TT + ti
                    emb = p1t.tile([P, D], F32, tag="emb")
                    nc.gpsimd.indirect_dma_start(
                        out=emb[:], out_offset=None, in_=t["tok_emb"][:],
                        in_offset=IndirectOffsetOnAxis(
                            ap=tok_idx_sb[:, g:g + 1], axis=0))
                    nc.gpsimd.indirect_dma_start(
                        out=emb[:], out_offset=None, in_=t["seg_emb"][:],
                        in_offset=IndirectOffsetOnAxis(
                            ap=seg_idx_sb[:, g:g + 1], axis=0),
                        compute_op=OP.add)
                    nc.gpsimd.dma_start(out=emb[:], in_=pos_r[ti],
                                        accum_op=OP.add)
                    # LayerNorm over d
                    stats = p1t.tile([P, 3, 6], F32, tag="stats")
                    for w in range(3):
                        nc.vector.bn_stats(out=stats[:, w, :],
                                           in_=emb[:, 256 * w:256 * (w + 1)])
                    mv = p1t.tile([P, 2], F32, tag="mv")
                    nc.vector.bn_aggr(out=mv[:], in_=stats[:])
                    rstd = p1t.tile([P, 1], F32, tag="rstd")
                    nc.scalar.activation(out=rstd[:], in_=mv[:, 1:2],
                                         func=AF.Sqrt, bias=eps_sb[:])
                    nc.vector.reciprocal(out=rstd[:], in_=rstd[:])
                    xln = p1t.tile([P, D], F32, tag="xln")
                    nc.vector.tensor_scalar(
                        out=xln[:], in0=emb[:], scalar1=mv[:, 0:1],
                        scalar2=rstd[:], op0=OP.subtract, op1=OP.mult)
                    nc.vector.tensor_mul(xln[:], xln[:], g_b[:])
                    nc.gpsimd.tensor_tensor(
                        out=xln[:], in0=xln[:], in1=b_b[:], op=OP.add)
                    for j in range(KD):
                        pst = ps1.tile([P, P], F32, tag="tr")
                        nc.tensor.transpose(pst[:], xln[:, P * j:P * (j + 1)],
                                            ident[:])
                        nc.scalar.copy(out=xT[:, j, P * ti:P * (ti + 1)],
                                       in_=pst[:])

                # q/k projections: per head pair [128, 512]
                for pr in range(NPAIR):
                    psq = psqk.tile([P, T], F32, tag="qk")
                    for ki in range(KD):
                        nc.tensor.matmul(
                            psq[:],
                            lhsT=wq_sb[:, ki, P * pr:P * (pr + 1)],
                            rhs=xT[:, ki, :],
                            start=(ki == 0), stop=(ki == KD - 1))
                    # fold query mask * 1/8 into the copy-out
                    nc.vector.tensor_mul(q_sb[:, b, pr, :], psq[:], mask_b[:, b, :])
                    psk = psqk.tile([P, T], F32, tag="qk")
                    for ki in range(KD):
                        nc.tensor.matmul(
                            psk[:],
                            lhsT=wk_sb[:, ki, P * pr:P * (pr + 1)],
                            rhs=xT[:, ki, :],
                            start=(ki == 0), stop=(ki == KD - 1))
                    nc.scalar.copy(out=k_sb[:, b, pr, :], in_=psk[:])

                # v natural [t, h*hs] + ones columns
                for ti in range(TT):
                    pv = psv.tile([P, D], F32, tag="v")
                    for ki in range(KD):
                        nc.tensor.matmul(
                            pv[:, 0:512],
                            lhsT=xT[:, ki, P * ti:P * (ti + 1)],
                            rhs=wv_sb[:, ki, 0:512],
                            start=(ki == 0), stop=(ki == KD - 1))
                    for ki in range(KD):
                        nc.tensor.matmul(
                            pv[:, 512:768],
                            lhsT=xT[:, ki, P * ti:P * (ti + 1)],
                            rhs=wv_sb[:, ki, 512:768],
                            start=(ki == 0), stop=(ki == KD - 1))
                    vv = v_sb[:, b, ti, :].rearrange("p (h s) -> p h s", s=HS + 1)
                    nc.vector.memset(v_sb[:, b, ti, :], 1.0)
                    nc.vector.tensor_copy(
                        out=vv[:, :, 0:HS],
                        in_=pv[:].rearrange("p (h s) -> p h s", s=HS))

        # ------------- phase 2+3: attention, select, allgather, heads ----
        with tc.tile_pool(name="att", bufs=2) as attp, \
             tc.tile_pool(name="attr", bufs=8) as attrp, \
             tc.tile_pool(name="xm", bufs=1) as xmp, \
             tc.tile_pool(name="lmw", bufs=2) as lmwp, \
             tc.tile_pool(name="lmc", bufs=1) as lmcp, \
             tc.tile_pool(name="lmo", bufs=3) as lmop, \
             tc.tile_pool(name="pss", bufs=4, space="PSUM") as pss_p, \
             tc.tile_pool(name="psa", bufs=4, space="PSUM") as psa_p:

            for b in range(BPC):
                for h in range(H):
                    pr, off = h // 2, HS * (h % 2)
                    pt = attp.tile([P, TT, T], F32, tag="pt")
                    for u in range(TT):
                        pss = pss_p.tile([P, T], F32, tag="s")
                        nc.tensor.matmul(
                            pss[:],
                            lhsT=k_sb[off:off + HS, b, pr,
                                      P * u:P * (u + 1)],
                            rhs=q_sb[off:off + HS, b, pr, :],
                            start=True, stop=True)
                        nc.scalar.activation(out=pt[:, u, :], in_=pss[:],
                                             func=AF.Exp)
                    vv = v_sb[:, b, :, :].rearrange("p ti (h s) -> p ti h s",
                                                    s=HS + 1)
                    for ti in range(TT):
                        psa = psa_p.tile([P, HS + 1], F32, tag="pv")
                        for u in range(TT):
                            nc.tensor.matmul(
                                psa[:], lhsT=pt[:, u, P * ti:P * (ti + 1)],
                                rhs=vv[:, u, h, :],
                                start=(u == 0), stop=(u == TT - 1))
                        rcp = attrp.tile([P, 1], F32, tag="rcp")
                        nc.vector.reciprocal(out=rcp[:], in_=psa[:, HS:HS + 1])
                        nc.vector.tensor_scalar_mul(
                            out=x_att[:, b, ti, HS * h:HS * (h + 1)],
                            in0=psa[:, 0:HS], scalar1=rcp[:])

            # ---- select masked positions (+ cls col) via one-hot matmul ----
            xm_sb = xmp.tile([P, BPC, KD, NM1], F32)
            for b in range(BPC):
                for j in range(KD):
                    psx = psa_p.tile([P, NM1], F32, tag="pv")
                    for ti in range(TT):
                        nc.tensor.matmul(
                            psx[:], lhsT=x_att[:, b, ti, P * j:P * (j + 1)],
                            rhs=sel_sb[:, b, ti, :],
                            start=(ti == 0), stop=(ti == TT - 1))
                    nc.scalar.copy(out=xm_sb[:, b, j, :], in_=psx[:])

            # cls head (local batches only)
            for b in range(BPC):
                psc = psa_p.tile([1, 2], F32, tag="pv")
                for j in range(KD):
                    nc.tensor.matmul(
                        psc[:], lhsT=xm_sb[:, b, j, NMASK:NM1],
                        rhs=w_cls_sb[:, j, :],
                        start=(j == 0), stop=(j == KD - 1))
                ocls = xmp.tile([1, 2], F32, tag="ocls")
                nc.vector.tensor_add(ocls[:], psc[:], b_cls_sb[:])
                nc.sync.dma_start(t["out_cls"][b:b + 1, :], ocls[:])

            # allgather masked-position features
            ag_in = dramp.tile([BPC, KD, P, NM1], F32)
            ag_out = dramp.tile([N_CORES, BPC, KD, P, NM1], F32)
            nc.sync.dma_start(ag_in[:].rearrange("b c p m -> p (b c) m"),
                              xm_sb[:].rearrange("p b c m -> p (b c) m"))
            if spmd:
                nc.gpsimd.collective_compute(
                    "AllGather", mybir.AluOpType.bypass,
                    replica_groups=[list(range(N_CORES))],
                    ins=[ag_in.opt()], outs=[ag_out.opt()])
            else:
                # timing-only stand-in for the AllGather: replicate the local
                # contribution so downstream reads see equivalent traffic
                for r in range(N_CORES):
                    nc.sync.dma_start(ag_out[r], ag_in[:])

            xg_sb = lmcp.tile([P, KD, MROWS], F32R)
            agv = ag_out[:].rearrange("r b c p m -> (r b) c p m")
            for j in range(KD):
                nc.gpsimd.dma_start(
                    out=xg_sb[:, j, :].rearrange("p (g m) -> p g m", m=NMASK),
                    in_=agv[:, j, :, 0:NMASK].rearrange("g p m -> p g m"))

            # lm head over local vocab shard
            blm_b = lmcp.tile([P, VS], F32)
            nc.sync.dma_start(blm_b[:], t["b_lm"][:].to_broadcast([P, VS]))
            wlm_r = t["w_lm"].rearrange("(ko p) v -> p ko v", p=P)
            voff = 0
            for vt, nv in enumerate(VT_SIZES):
                wt = lmwp.tile([P, KD, 512], F32R, tag="wlm")
                nc.gpsimd.dma_start(out=wt[:, :, 0:nv], in_=wlm_r[:, :, voff:voff + nv])
                for mt in range(3):
                    mp = min(P, MROWS - P * mt)
                    pso = pss_p.tile([P, 512], F32, tag="s")
                    for j in range(KD):
                        nc.tensor.matmul(
                            pso[:mp, 0:nv],
                            lhsT=xg_sb[:, j, P * mt:P * mt + mp],
                            rhs=wt[:, j, 0:nv],
                            start=(j == 0), stop=(j == KD - 1))
                    osb = lmop.tile([P, 512], F32, tag="osb")
                    nc.vector.tensor_add(osb[:mp, 0:nv], pso[:mp, 0:nv],
                                         blm_b[:mp, voff:voff + nv])
                    nc.sync.dma_start(
                        t["out_lm"][P * mt:P * mt + mp, voff:voff + nv],
                        osb[:mp, 0:nv])
                voff += nv


_CACHE = {}


def _get_program():
    if "nc" not in _CACHE:
        _CACHE["nc"] = build_program()
    return _CACHE["nc"]


def _prep_inputs(inputs):
    seq = np.ascontiguousarray(np.asarray(inputs["sequence"], np.int64)).astype(np.int32)
    seg = np.ascontiguousarray(np.asarray(inputs["segment"], np.int64)).astype(np.int32)
    amask = np.asarray(inputs["attn_mask"]).astype(np.float32)
    mpos = np.asarray(inputs["masked_pos"], np.int64).astype(np.int64)
    f = lambda k: np.ascontiguousarray(np.asarray(inputs[k], np.float32))
    tok_emb, seg_emb, pos_emb = f("tok_emb"), f("seg_emb"), f("pos_emb")
    ln_g, ln_b = f("ln_g").reshape(1, D), f("ln_b").reshape(1, D)
    Wq, Wk, Wv = f("Wq"), f("Wk"), f("Wv")
    W_lm, b_lm = f("W_lm"), f("b_lm").reshape(1, V)
    W_cls, b_cls = f("W_cls"), f("b_cls").reshape(1, 2)

    wq_flat = np.ascontiguousarray(Wq.transpose(1, 0, 2).reshape(D, D))
    wk_flat = np.ascontiguousarray(Wk.transpose(1, 0, 2).reshape(D, D))
    wv_flat = np.ascontiguousarray(Wv.transpose(1, 0, 2).reshape(D, D))
    # [768, 2] -> [6, 128, 2] -> [128, 6, 2] -> [128, 12]
    w_cls_dev = np.ascontiguousarray(
        W_cls.reshape(KD, P, 2).transpose(1, 0, 2).reshape(P, KD * 2))

    in_maps = []
    for c in range(N_CORES):
        b0 = BPC * c
        # [2, 512] -> [2*4, 128] -> [128, 8]
        tok_idx = np.ascontiguousarray(
            seq[b0:b0 + BPC].reshape(BPC * TT, P).T)
        seg_idx = np.ascontiguousarray(
            seg[b0:b0 + BPC].reshape(BPC * TT, P).T)
        mask_sc = np.ascontiguousarray(amask[b0:b0 + BPC] * np.float32(HS ** -0.5))
        sel = np.zeros((BPC, T, NM1), np.float32)
        for b in range(BPC):
            sel[b, mpos[b0 + b], np.arange(NMASK)] = 1.0
            sel[b, 0, NMASK] = 1.0
        # [2, 512, 21] -> [2, 4, 128, 21] -> [128, 2, 4, 21] -> [128, 168]
        sel_dev = np.ascontiguousarray(
            sel.reshape(BPC, TT, P, NM1).transpose(2, 0, 1, 3).reshape(P, -1))
        in_maps.append({
            "tok_idx": tok_idx, "seg_idx": seg_idx, "mask_sc": mask_sc,
            "sel": sel_dev, "pos_emb": pos_emb, "tok_emb": tok_emb,
            "seg_emb": seg_emb, "ln_g": ln_g, "ln_b": ln_b,
            "wq": wq_flat, "wk": wk_flat, "wv": wv_flat,
            "w_lm": np.ascontiguousarray(W_lm[:, VS * c:VS * (c + 1)]),
            "b_lm": np.ascontiguousarray(b_lm[:, VS * c:VS * (c + 1)]),
            "w_cls": w_cls_dev, "b_cls": b_cls,
        })
    return in_maps


def kernel(**inputs):
    nc = _get_program()
    in_maps = _prep_inputs(inputs)
    res = run_bass_kernel_spmd(nc, in_maps, core_ids=list(range(N_CORES)))
    logits_lm = np.empty((B, NMASK, V), np.float32)
    logits_clsf = np.empty((B, 2), np.float32)
    for c in range(N_CORES):
        logits_lm[:, :, VS * c:VS * (c + 1)] = \
            res.results[c]["out_lm"].reshape(B, NMASK, VS)
        logits_clsf[BPC * c:BPC * (c + 1)] = res.results[c]["out_cls"]
    return (logits_lm, logits_clsf)


# revision 20
# speedup vs baseline: 2.1504x; 1.1207x over previous
"""BERT-style single-layer transformer on 8 Trainium2 NeuronCores.

Sharding: data-parallel over batch (2 batches/core) for embedding + LN +
attention; the lm_head is vocab-sharded (4000 vocab cols/core) with an
on-device AllGather of the gathered masked positions.

Per-core device program (all fp32):
  - token embedding via indirect-DMA row gather; segment embedding gathered
    with DMA-accumulate; position embedding DMA-accumulated on top
  - LayerNorm (bn_stats) in natural [t, d] layout, PE transpose to
    xT [d-chunk, t]
  - QKV: qT/kT per head-pair [128, 512], v natural [t, 12*65] with a ones
    column per head (so the PV matmul also produces softmax denominators)
  - scores S^T = k @ q^T in [key, query] layout; query mask (*1/8) folded
    into the qT PSUM->SBUF copy; max-free softmax via ACT Exp (scores are
    tiny: layernormed x times 0.02-scale weights)
  - PV natural + per-partition normalize -> x_att [t, d]
  - one-hot selection matmul -> xmT [d, 21] per batch (masked pos + cls)
  - AllGather xmT across 8 cores, lm_head over the local vocab shard
"""

import numpy as np

import concourse.bass as bass
import concourse.tile as tile
from concourse import bacc, mybir
from concourse.bass import IndirectOffsetOnAxis
from concourse.bass_utils import run_bass_kernel_spmd
from concourse.masks import make_identity

F32 = mybir.dt.float32
F32R = mybir.dt.float32r
I32 = mybir.dt.int32

V, NSEG, MAXLEN, D, H = 32000, 2, 512, 768, 12
HS = D // H                     # 64
B, T, NMASK = 16, 512, 20
LN_EPS = 1e-5
N_CORES = 8
BPC = B // N_CORES              # batches per core = 2
VS = V // N_CORES               # vocab shard = 4000
P = 128
KD = D // P                     # d chunks = 6
TT = T // P                     # t tiles per batch = 4
NPAIR = H // 2                  # head pairs = 6
NM1 = NMASK + 1                 # masked positions + cls column = 21
MROWS = B * NMASK               # 320 lm rows
VT_SIZES = [512] * 7 + [VS - 512 * 7]   # vocab tiles per core


def build_program(spmd=True):
    nc = bacc.Bacc("TRN2", target_bir_lowering=False, debug=False,
                   num_devices=N_CORES if spmd else 1)

    def din(name, shape, dtype=F32):
        return nc.dram_tensor(name, shape, dtype, kind="ExternalInput").ap()

    tok_idx = din("tok_idx", [P, BPC * TT], I32)     # col g = b*TT+ti
    seg_f = din("seg_f", [P, BPC * TT])              # segment id as f32
    mask_sc = din("mask_sc", [BPC, T])               # attn_mask * 1/8
    sel = din("sel", [P, BPC * TT * NM1])            # one-hot select [p, b, ti, m]
    pos_emb = din("pos_emb", [MAXLEN, D])            # pos_emb + seg_emb[0]
    tok_emb = din("tok_emb", [V, D])
    seg_d = din("seg_d", [1, D])                     # seg_emb[1] - seg_emb[0]
    ln_g = din("ln_g", [1, D])
    ln_b = din("ln_b", [1, D])
    wq = din("wq", [D, D], F32R)                     # [d, h*hs]
    wk = din("wk", [D, D], F32R)
    wv = din("wv", [D, D], F32R)
    w_lm = din("w_lm", [D, VS], F32R)
    b_lm = din("b_lm", [1, VS])
    w_cls = din("w_cls", [P, KD * 2], F32R)          # prearranged [p, ko, 2]
    b_cls = din("b_cls", [1, 2])

    out_lm = nc.dram_tensor("out_lm", [MROWS, VS], F32, kind="ExternalOutput").ap()
    out_cls = nc.dram_tensor("out_cls", [BPC, 2], F32, kind="ExternalOutput").ap()

    with tile.TileContext(nc) as tc:
        _emit(tc, locals(), spmd)
    nc.compile()
    return nc


def _emit(tc, t, spmd=True):
    nc = tc.nc
    AF = mybir.ActivationFunctionType
    OP = mybir.AluOpType

    with tc.tile_pool(name="const", bufs=1) as constp, \
         tc.tile_pool(name="attout", bufs=1) as attoutp, \
         tc.tile_pool(name="dram", bufs=1, space="DRAM") as dramp:

        # ---------------- constants ----------------
        ident = constp.tile([P, P], F32)
        make_identity(nc, ident[:])
        tok_idx_sb = constp.tile([P, BPC * TT], I32)
        nc.sync.dma_start(tok_idx_sb[:], t["tok_idx"][:])
        seg_f_sb = constp.tile([P, BPC * TT], F32)
        nc.sync.dma_start(seg_f_sb[:], t["seg_f"][:])
        seg_d_b = constp.tile([P, D], F32)
        nc.sync.dma_start(seg_d_b[:], t["seg_d"][:].to_broadcast([P, D]))
        mask_b = constp.tile([P, BPC, T], F32)
        for b in range(BPC):
            nc.sync.dma_start(mask_b[:, b, :],
                              t["mask_sc"][b:b + 1, :].to_broadcast([P, T]))
        sel_sb = constp.tile([P, BPC, TT, NM1], F32)
        nc.sync.dma_start(
            sel_sb[:].rearrange("p b ti m -> p (b ti m)"), t["sel"][:])
        g_b = constp.tile([P, D], F32)
        nc.sync.dma_start(g_b[:], t["ln_g"][:].to_broadcast([P, D]))
        b_b = constp.tile([P, D], F32)
        nc.sync.dma_start(b_b[:], t["ln_b"][:].to_broadcast([P, D]))
        eps_sb = constp.tile([P, 1], F32)
        nc.vector.memset(eps_sb[:], LN_EPS)
        w_cls_sb = constp.tile([P, KD, 2], F32R)
        nc.sync.dma_start(w_cls_sb[:].rearrange("p k c -> p (k c)"), t["w_cls"][:])
        b_cls_sb = constp.tile([1, 2], F32)
        nc.sync.dma_start(b_cls_sb[:], t["b_cls"][:])

        x_att = attoutp.tile([P, BPC, TT, D], F32)

        wq_r = t["wq"].rearrange("(ko p) m -> p ko m", p=P)
        wk_r = t["wk"].rearrange("(ko p) m -> p ko m", p=P)
        wv_r = t["wv"].rearrange("(ko p) m -> p ko m", p=P)
        pos_r = t["pos_emb"].rearrange("(ti p) d -> ti p d", p=P)

        with tc.tile_pool(name="qkv", bufs=1) as qkvp:
            q_sb = qkvp.tile([P, BPC, NPAIR, T], F32R)
            k_sb = qkvp.tile([P, BPC, NPAIR, T], F32R)
            v_sb = qkvp.tile([P, BPC, TT, H * (HS + 1)], F32)  # +ones col/head

            # ------------ phase 1: embed + LN + transpose + QKV ----------
            with tc.tile_pool(name="p1w", bufs=1) as p1w, \
                 tc.tile_pool(name="p1t", bufs=3) as p1t, \
                 tc.tile_pool(name="ps1", bufs=2, space="PSUM") as ps1, \
                 tc.tile_pool(name="psqk", bufs=2, space="PSUM") as psqk, \
                 tc.tile_pool(name="psv", bufs=2, space="PSUM") as psv:

                wq_sb = p1w.tile([P, KD, D], F32R)
                nc.sync.dma_start(wq_sb[:], wq_r)
                wk_sb = p1w.tile([P, KD, D], F32R)
                nc.sync.dma_start(wk_sb[:], wk_r)
                wv_sb = p1w.tile([P, KD, D], F32R)
                nc.sync.dma_start(wv_sb[:], wv_r)

                for b in range(BPC):
                    xT = p1w.tile([P, KD, T], F32R, tag="xT")
                    for ti in range(TT):
                        g = b * TT + ti
                        emb = p1t.tile([P, D], F32, tag="emb")
                        nc.gpsimd.indirect_dma_start(
                            out=emb[:], out_offset=None, in_=t["tok_emb"][:],
                            in_offset=IndirectOffsetOnAxis(
                                ap=tok_idx_sb[:, g:g + 1], axis=0))
                        nc.gpsimd.dma_start(out=emb[:], in_=pos_r[ti],
                                            accum_op=OP.add)
                        # segment embedding: emb += seg_id * (seg1 - seg0)
                        segt = p1t.tile([P, D], F32, tag="segt")
                        nc.gpsimd.tensor_scalar_mul(
                            out=segt[:], in0=seg_d_b[:],
                            scalar1=seg_f_sb[:, g:g + 1])
                        nc.vector.tensor_add(emb[:], emb[:], segt[:])
                        # LayerNorm over d
                        stats = p1t.tile([P, 3, 6], F32, tag="stats")
                        for w in range(3):
                            nc.vector.bn_stats(out=stats[:, w, :],
                                               in_=emb[:, 256 * w:256 * (w + 1)])
                        mv = p1t.tile([P, 2], F32, tag="mv")
                        nc.vector.bn_aggr(out=mv[:], in_=stats[:])
                        rstd = p1t.tile([P, 1], F32, tag="rstd")
                        nc.scalar.activation(out=rstd[:], in_=mv[:, 1:2],
                                             func=AF.Sqrt, bias=eps_sb[:])
                        nc.vector.reciprocal(out=rstd[:], in_=rstd[:])
                        xln = p1t.tile([P, D], F32, tag="segt")
                        nc.vector.tensor_scalar(
                            out=xln[:], in0=emb[:], scalar1=mv[:, 0:1],
                            scalar2=rstd[:], op0=OP.subtract, op1=OP.mult)
                        nc.vector.tensor_mul(xln[:], xln[:], g_b[:])
                        nc.gpsimd.tensor_tensor(
                            out=xln[:], in0=xln[:], in1=b_b[:], op=OP.add)
                        for j in range(KD):
                            pst = ps1.tile([P, P], F32, tag="tr")
                            nc.tensor.transpose(
                                pst[:], xln[:, P * j:P * (j + 1)], ident[:])
                            nc.scalar.copy(out=xT[:, j, P * ti:P * (ti + 1)],
                                           in_=pst[:])

                    # q/k projections: per head pair [128, 512]
                    for pr in range(NPAIR):
                        psq = psqk.tile([P, T], F32, tag="qk")
                        for ki in range(KD):
                            nc.tensor.matmul(
                                psq[:], lhsT=wq_sb[:, ki, P * pr:P * (pr + 1)],
                                rhs=xT[:, ki, :],
                                start=(ki == 0), stop=(ki == KD - 1))
                        # fold query mask * 1/8 into the copy-out
                        nc.vector.tensor_mul(q_sb[:, b, pr, :], psq[:],
                                             mask_b[:, b, :])
                        psk = psqk.tile([P, T], F32, tag="qk")
                        for ki in range(KD):
                            nc.tensor.matmul(
                                psk[:], lhsT=wk_sb[:, ki, P * pr:P * (pr + 1)],
                                rhs=xT[:, ki, :],
                                start=(ki == 0), stop=(ki == KD - 1))
                        nc.scalar.copy(out=k_sb[:, b, pr, :], in_=psk[:])

                    # v natural [t, h*hs] + ones columns
                    for ti in range(TT):
                        pv = psv.tile([P, D], F32, tag="v")
                        for ki in range(KD):
                            nc.tensor.matmul(
                                pv[:, 0:512], lhsT=xT[:, ki, P * ti:P * (ti + 1)],
                                rhs=wv_sb[:, ki, 0:512],
                                start=(ki == 0), stop=(ki == KD - 1))
                        for ki in range(KD):
                            nc.tensor.matmul(
                                pv[:, 512:768],
                                lhsT=xT[:, ki, P * ti:P * (ti + 1)],
                                rhs=wv_sb[:, ki, 512:768],
                                start=(ki == 0), stop=(ki == KD - 1))
                        vv = v_sb[:, b, ti, :].rearrange("p (h s) -> p h s",
                                                         s=HS + 1)
                        nc.vector.memset(v_sb[:, b, ti, :], 1.0)
                        nc.vector.tensor_copy(
                            out=vv[:, :, 0:HS],
                            in_=pv[:].rearrange("p (h s) -> p h s", s=HS))

            # ---------------- phase 2: attention ----------------
            with tc.tile_pool(name="att", bufs=2) as attp, \
                 tc.tile_pool(name="attr", bufs=8) as attrp, \
                 tc.tile_pool(name="pss", bufs=4, space="PSUM") as pss_p, \
                 tc.tile_pool(name="psa", bufs=4, space="PSUM") as psa_p:
                for b in range(BPC):
                    for h in range(H):
                        pr, off = h // 2, HS * (h % 2)
                        pt = attp.tile([P, TT, T], F32, tag="pt")
                        for u in range(TT):
                            pss = pss_p.tile([P, T], F32, tag="s")
                            nc.tensor.matmul(
                                pss[:],
                                lhsT=k_sb[off:off + HS, b, pr,
                                          P * u:P * (u + 1)],
                                rhs=q_sb[off:off + HS, b, pr, :],
                                start=True, stop=True)
                            nc.scalar.activation(out=pt[:, u, :], in_=pss[:],
                                                 func=AF.Exp)
                        vv = v_sb[:, b, :, :].rearrange(
                            "p ti (h s) -> p ti h s", s=HS + 1)
                        for ti in range(TT):
                            psa = psa_p.tile([P, HS + 1], F32, tag="pv")
                            for u in range(TT):
                                nc.tensor.matmul(
                                    psa[:], lhsT=pt[:, u, P * ti:P * (ti + 1)],
                                    rhs=vv[:, u, h, :],
                                    start=(u == 0), stop=(u == TT - 1))
                            rcp = attrp.tile([P, 1], F32, tag="rcp")
                            nc.vector.reciprocal(out=rcp[:],
                                                 in_=psa[:, HS:HS + 1])
                            nc.vector.tensor_scalar_mul(
                                out=x_att[:, b, ti, HS * h:HS * (h + 1)],
                                in0=psa[:, 0:HS], scalar1=rcp[:])

        # ------- phase 3: select + allgather + cls + lm head -------------
        with tc.tile_pool(name="xm", bufs=1) as xmp, \
             tc.tile_pool(name="lmw", bufs=3) as lmwp, \
             tc.tile_pool(name="lmc", bufs=1) as lmcp, \
             tc.tile_pool(name="lmo", bufs=3) as lmop, \
             tc.tile_pool(name="psx", bufs=4, space="PSUM") as psx_p, \
             tc.tile_pool(name="pso", bufs=4, space="PSUM") as pso_p:

            xm_sb = xmp.tile([P, BPC, KD, NM1], F32R)
            for b in range(BPC):
                for j in range(KD):
                    psx = psx_p.tile([P, NM1], F32, tag="x")
                    for ti in range(TT):
                        nc.tensor.matmul(
                            psx[:], lhsT=x_att[:, b, ti, P * j:P * (j + 1)],
                            rhs=sel_sb[:, b, ti, :],
                            start=(ti == 0), stop=(ti == TT - 1))
                    nc.scalar.copy(out=xm_sb[:, b, j, :], in_=psx[:])

            # cls head (local batches only)
            for b in range(BPC):
                psc = psx_p.tile([1, 2], F32, tag="x")
                for j in range(KD):
                    nc.tensor.matmul(
                        psc[:], lhsT=xm_sb[:, b, j, NMASK:NM1],
                        rhs=w_cls_sb[:, j, :],
                        start=(j == 0), stop=(j == KD - 1))
                ocls = xmp.tile([1, 2], F32, tag="ocls")
                nc.vector.tensor_add(ocls[:], psc[:], b_cls_sb[:])
                nc.sync.dma_start(t["out_cls"][b:b + 1, :], ocls[:])

            # allgather masked-position features (f32r end to end)
            ag_in = dramp.tile([BPC, KD, P, NM1], F32R)
            ag_out = dramp.tile([N_CORES, BPC, KD, P, NM1], F32R)
            nc.sync.dma_start(ag_in[:].rearrange("b c p m -> p (b c) m"),
                              xm_sb[:].rearrange("p b c m -> p (b c) m"))
            if spmd:
                nc.gpsimd.collective_compute(
                    "AllGather", mybir.AluOpType.bypass,
                    replica_groups=[list(range(N_CORES))],
                    ins=[ag_in.opt()], outs=[ag_out.opt()])
            else:
                # timing-only stand-in for the AllGather
                for r in range(N_CORES):
                    nc.sync.dma_start(ag_out[r], ag_in[:])

            xg_sb = lmcp.tile([P, KD, MROWS], F32R)
            agv = ag_out[:].rearrange("r b c p m -> (r b) c p m")
            for j in range(KD):
                nc.sync.dma_start(
                    xg_sb[:, j, :].rearrange("p (g m) -> p g m", m=NMASK),
                    agv[:, j, :, 0:NMASK].rearrange("g p m -> p g m"))

            # lm head over local vocab shard
            blm_b = lmcp.tile([P, VS], F32)
            nc.sync.dma_start(blm_b[:], t["b_lm"][:].to_broadcast([P, VS]))
            wlm_r = t["w_lm"].rearrange("(ko p) v -> p ko v", p=P)
            voff = 0
            for vt, nv in enumerate(VT_SIZES):
                wt = lmwp.tile([P, KD, 512], F32R, tag="wlm")
                nc.sync.dma_start(wt[:, :, 0:nv], wlm_r[:, :, voff:voff + nv])
                for mt in range(3):
                    mp = min(P, MROWS - P * mt)
                    pso = pso_p.tile([P, 512], F32, tag="o")
                    for j in range(KD):
                        nc.tensor.matmul(
                            pso[:mp, 0:nv],
                            lhsT=xg_sb[:, j, P * mt:P * mt + mp],
                            rhs=wt[:, j, 0:nv],
                            start=(j == 0), stop=(j == KD - 1))
                    osb = lmop.tile([P, 512], F32, tag="osb")
                    nc.vector.tensor_add(osb[:mp, 0:nv], pso[:mp, 0:nv],
                                         blm_b[:mp, voff:voff + nv])
                    nc.sync.dma_start(
                        t["out_lm"][P * mt:P * mt + mp, voff:voff + nv],
                        osb[:mp, 0:nv])
                voff += nv


_CACHE = {}


def _get_program():
    if "nc" not in _CACHE:
        _CACHE["nc"] = build_program()
    return _CACHE["nc"]


def _prep_inputs(inputs):
    seq = np.ascontiguousarray(np.asarray(inputs["sequence"], np.int64)).astype(np.int32)
    seg = np.ascontiguousarray(np.asarray(inputs["segment"], np.int64)).astype(np.int32)
    amask = np.asarray(inputs["attn_mask"]).astype(np.float32)
    mpos = np.asarray(inputs["masked_pos"], np.int64).astype(np.int64)
    f = lambda k: np.ascontiguousarray(np.asarray(inputs[k], np.float32))
    tok_emb, seg_emb, pos_emb = f("tok_emb"), f("seg_emb"), f("pos_emb")
    ln_g, ln_b = f("ln_g").reshape(1, D), f("ln_b").reshape(1, D)
    Wq, Wk, Wv = f("Wq"), f("Wk"), f("Wv")
    W_lm, b_lm = f("W_lm"), f("b_lm").reshape(1, V)
    W_cls, b_cls = f("W_cls"), f("b_cls").reshape(1, 2)

    wq_flat = np.ascontiguousarray(Wq.transpose(1, 0, 2).reshape(D, D))
    wk_flat = np.ascontiguousarray(Wk.transpose(1, 0, 2).reshape(D, D))
    wv_flat = np.ascontiguousarray(Wv.transpose(1, 0, 2).reshape(D, D))
    # [768, 2] -> [6, 128, 2] -> [128, 6, 2] -> [128, 12]
    w_cls_dev = np.ascontiguousarray(
        W_cls.reshape(KD, P, 2).transpose(1, 0, 2).reshape(P, KD * 2))

    pos2 = np.ascontiguousarray(pos_emb + seg_emb[0][None, :])
    seg_d = np.ascontiguousarray((seg_emb[1] - seg_emb[0]).reshape(1, D))

    in_maps = []
    for c in range(N_CORES):
        b0 = BPC * c
        # [2, 512] -> [2*4, 128] -> [128, 8]
        tok_idx = np.ascontiguousarray(
            seq[b0:b0 + BPC].reshape(BPC * TT, P).T)
        seg_f = np.ascontiguousarray(
            seg[b0:b0 + BPC].reshape(BPC * TT, P).T.astype(np.float32))
        mask_sc = np.ascontiguousarray(amask[b0:b0 + BPC] * np.float32(HS ** -0.5))
        sel = np.zeros((BPC, T, NM1), np.float32)
        for b in range(BPC):
            sel[b, mpos[b0 + b], np.arange(NMASK)] = 1.0
            sel[b, 0, NMASK] = 1.0
        # [2, 512, 21] -> [2, 4, 128, 21] -> [128, 2, 4, 21] -> [128, 168]
        sel_dev = np.ascontiguousarray(
            sel.reshape(BPC, TT, P, NM1).transpose(2, 0, 1, 3).reshape(P, -1))
        in_maps.append({
            "tok_idx": tok_idx, "seg_f": seg_f, "mask_sc": mask_sc,
            "sel": sel_dev, "pos_emb": pos2, "tok_emb": tok_emb,
            "seg_d": seg_d, "ln_g": ln_g, "ln_b": ln_b,
            "wq": wq_flat, "wk": wk_flat, "wv": wv_flat,
            "w_lm": np.ascontiguousarray(W_lm[:, VS * c:VS * (c + 1)]),
            "b_lm": np.ascontiguousarray(b_lm[:, VS * c:VS * (c + 1)]),
            "w_cls": w_cls_dev, "b_cls": b_cls,
        })
    return in_maps


def kernel(**inputs):
    nc = _get_program()
    in_maps = _prep_inputs(inputs)
    res = run_bass_kernel_spmd(nc, in_maps, core_ids=list(range(N_CORES)))
    logits_lm = np.empty((B, NMASK, V), np.float32)
    logits_clsf = np.empty((B, 2), np.float32)
    for c in range(N_CORES):
        logits_lm[:, :, VS * c:VS * (c + 1)] = \
            res.results[c]["out_lm"].reshape(B, NMASK, VS)
        logits_clsf[BPC * c:BPC * (c + 1)] = res.results[c]["out_cls"]
    return (logits_lm, logits_clsf)
